# revision 6
# baseline (speedup 1.0000x reference)
"""Trainium2 Bass kernel for nn_GCNModelCMVAE (GCN encoder + inner-product decoder).

Self-contained: hardcodes shapes/sharding. Strategy (8 NeuronCores, row-sharded):

  L1: per-core  XW0_shard = featT_shard.T @ W0              [1024, 32]
      (host passes features pre-transposed; pure layout prep)
  -- host gathers XW0 shards -> XW0_full [8192, 32]
  L2: per-core  h1_shard = relu(A_shard @ XW0) via dense bf16 matmul against
      the host-materialized adjacency slice AT_shard [8192, 1024] (A_shard.T).
      Zeros are exact in bf16 so this equals an edge-wise scatter numerically;
      lhsT = AT chunk [128src, 128dst], rhs = XW0 chunk [128src, 32].
  -- host gathers h1 shards -> h1_full [8192, 32]
  L3: per-core  sT [32, 1024] = h1.T @ AT_shard  (lhsT = h1 chunk [128,32],
      rhs = AT chunk [128, 512]), then per 128-row block:
      zcat[128,48] = matmul(lhsT=sT[:,block], rhs=[W1|W2|W3]), softmax on
      cols 16:32 / 32:48 (free-dim reductions), reparam -> z_shard [1024, 16]
      (uses spmm(A, h1@Wi) == spmm(A, h1) @ Wi)
  -- host transposes z -> zT [16, 8192] (bf16)
  L4: per-core  decode rows: out[128,512] = matmul(lhsT=zT_my[16,128],
      rhs=zT[16,512chunk]) for 8x16 tiles; DMA 32 MiB/core to HBM
      (memory roofline: ~256 MiB total output write)
"""

import numpy as np
import ml_dtypes
from contextlib import ExitStack

import concourse.bass as bass
import concourse.tile as tile
from concourse import bacc, mybir
from concourse.bass_utils import run_bass_kernel_spmd

F32 = mybir.dt.float32
BF16 = mybir.dt.bfloat16
NPBF16 = ml_dtypes.bfloat16

N = 8192
F = 512
H1 = 32
H2 = 16
NCORES = 8
RS = N // NCORES          # 1024 rows per core
P = 128
NBLK = RS // P            # 8 row-blocks per core
KCH = F // P              # 4 contraction chunks for XW0
NCH = N // P              # 64 source chunks for the spmm
NGRP = 8                  # AT DMA groups (8 chunks = 2 MiB each)
NCOL = N // 512           # 16 column chunks in decode
CORE_IDS = list(range(NCORES))

_CACHE = {}


# --------------------------------------------------------------------------
# kernel builders
# --------------------------------------------------------------------------

def _build_l1():
    nc = bacc.Bacc("TRN2", target_bir_lowering=False, debug=False,
                   num_devices=NCORES)
    featT = nc.dram_tensor("featT", [F, RS], F32, kind="ExternalInput").ap()
    w0 = nc.dram_tensor("w0", [F, H1], F32, kind="ExternalInput").ap()
    xw0 = nc.dram_tensor("xw0", [RS, H1], F32, kind="ExternalOutput").ap()

    with tile.TileContext(nc) as tc, ExitStack() as ctx:
        sb = ctx.enter_context(tc.tile_pool(name="sb", bufs=1))
        ps = ctx.enter_context(tc.tile_pool(name="ps", bufs=4, space="PSUM"))

        w0_sb = sb.tile([P, KCH * H1], F32)
        for k in range(KCH):
            nc.sync.dma_start(w0_sb[:, bass.ts(k, H1)],
                              w0[k * P:(k + 1) * P, :])
        ft = []
        for k in range(KCH):
            t = sb.tile([P, RS], F32, tag=f"ft{k}")
            nc.sync.dma_start(t[:], featT[k * P:(k + 1) * P, :])
            ft.append(t)

        out_sb = sb.tile([P, NBLK * H1], F32)
        for m in range(NBLK):
            acc = ps.tile([P, H1], F32)
            for k in range(KCH):
                nc.tensor.matmul(acc[:], lhsT=ft[k][:, bass.ts(m, P)],
                                 rhs=w0_sb[:, bass.ts(k, H1)],
                                 start=(k == 0), stop=(k == KCH - 1))
            nc.vector.tensor_copy(out_sb[:, bass.ts(m, H1)], acc[:])
        nc.sync.dma_start(xw0.rearrange("(m p) h -> p m h", p=P), out_sb[:])
    nc.compile()
    return nc


def _build_spmm(is_l3):
    """L2: h1_shard = relu(A_shard @ x) row-major.
    L3: sT = (A_shard @ h1).T via transposed orientation, then the z tail.
    Both stream the dense AT_shard [8192, 1024] bf16 in NGRP groups."""
    nc = bacc.Bacc("TRN2", target_bir_lowering=False, debug=False,
                   num_devices=NCORES)
    src = nc.dram_tensor("src", [N, H1], F32, kind="ExternalInput").ap()
    at = nc.dram_tensor("at", [NCH, P, RS], BF16, kind="ExternalInput").ap()
    if is_l3:
        wcat = nc.dram_tensor("wcat", [H1, 3 * H2], F32, kind="ExternalInput").ap()
        s1 = nc.dram_tensor("s1", [RS, H2], F32, kind="ExternalInput").ap()
        s2 = nc.dram_tensor("s2", [RS, H2], F32, kind="ExternalInput").ap()
        z_bf = nc.dram_tensor("z_bf", [RS, H2], BF16, kind="ExternalOutput").ap()
        z_f32 = nc.dram_tensor("z_f32", [RS, H2], F32, kind="ExternalOutput").ap()
    else:
        h1 = nc.dram_tensor("h1", [RS, H1], F32, kind="ExternalOutput").ap()

    AF = mybir.ActivationFunctionType
    ch_per_grp = NCH // NGRP
    with tile.TileContext(nc) as tc, ExitStack() as ctx:
        sb = ctx.enter_context(tc.tile_pool(name="sb", bufs=1))
        atp = ctx.enter_context(tc.tile_pool(name="atp", bufs=2))
        work = ctx.enter_context(tc.tile_pool(name="work", bufs=3))
        small = ctx.enter_context(tc.tile_pool(name="small", bufs=4))
        ps = ctx.enter_context(tc.tile_pool(name="ps", bufs=1, space="PSUM"))
        ps2 = ctx.enter_context(tc.tile_pool(name="ps2", bufs=2, space="PSUM"))

        # x [8192, 32] f32 -> SBUF [128, 64, 32] (chunk n on cols ts(n, 32)), cast bf16
        x_f = sb.tile([P, NCH * H1], F32)
        nc.sync.dma_start(x_f[:], src.rearrange("(n p) f -> p n f", p=P))
        x_bf = sb.tile([P, NCH * H1], BF16)
        nc.vector.tensor_copy(x_bf[:], x_f[:])

        if is_l3:
            # accumulate sT [32, 1024] across all 64 chunks (2 psum halves)
            acc = [ps.tile([H1, 512], F32, tag=f"sT{h}", name=f"sT{h}") for h in range(2)]
        else:
            acc = [ps.tile([P, H1], F32, tag=f"h1_{m}", name=f"h1acc{m}") for m in range(NBLK)]

        for g in range(NGRP):
            at_g = atp.tile([P, ch_per_grp * RS], BF16)
            nc.sync.dma_start(at_g[:], at[g * ch_per_grp:(g + 1) * ch_per_grp]
                              .rearrange("c p r -> p c r"))
            for ci in range(ch_per_grp):
                n = g * ch_per_grp + ci
                first, last = (n == 0), (n == NCH - 1)
                if is_l3:
                    for h in range(2):
                        nc.tensor.matmul(
                            acc[h][:],
                            lhsT=x_bf[:, bass.ts(n, H1)],
                            rhs=at_g[:, ci * RS + h * 512: ci * RS + (h + 1) * 512],
                            start=first, stop=last)
                else:
                    for m in range(NBLK):
                        nc.tensor.matmul(
                            acc[m][:],
                            lhsT=at_g[:, ci * RS + m * P: ci * RS + (m + 1) * P],
                            rhs=x_bf[:, bass.ts(n, H1)],
                            start=first, stop=last)

        if not is_l3:
            out_sb = sb.tile([P, NBLK * H1], F32)
            for m in range(NBLK):
                nc.scalar.activation(out_sb[:, bass.ts(m, H1)], acc[m][:], AF.Relu)
            nc.sync.dma_start(h1.rearrange("(m p) h -> p m h", p=P), out_sb[:])
        else:
            st_sb = sb.tile([H1, RS], F32)
            for h in range(2):
                nc.vector.tensor_copy(st_sb[:, h * 512:(h + 1) * 512], acc[h][:])

            wcat_sb = sb.tile([H1, 3 * H2], F32)
            nc.sync.dma_start(wcat_sb[:], wcat[:])
            s1_sb = sb.tile([P, NBLK * H2], F32)
            nc.sync.dma_start(s1_sb[:], s1.rearrange("(b p) h -> p b h", p=P))
            s2_sb = sb.tile([P, NBLK * H2], F32)
            nc.sync.dma_start(s2_sb[:], s2.rearrange("(b p) h -> p b h", p=P))
            zall_bf = sb.tile([P, NBLK * H2], BF16)
            zall_f32 = sb.tile([P, NBLK * H2], F32)

            # zcat for all 8 blocks into one psum tile [128, 8*48] (1.5 KB/bank)
            zps = ps2.tile([P, NBLK * 3 * H2], F32)
            for b in range(NBLK):
                nc.tensor.matmul(zps[:, bass.ts(b, 3 * H2)],
                                 lhsT=st_sb[:, bass.ts(b, P)],
                                 rhs=wcat_sb[:], start=True, stop=True)
            zc = sb.tile([P, NBLK * 3 * H2], F32)
            nc.vector.tensor_copy(zc[:], zps[:])
            zc3 = zc[:].rearrange("p (b j) -> p b j", j=3 * H2)

            # all-blocks-fused softmax -> exp for segs 1 (z_en) and 2 (z_he)
            ez = []
            for j in (1, 2):
                seg = zc3[:, :, j * H2:(j + 1) * H2]        # [P, 8, 16]
                mx = small.tile([P, NBLK], F32, tag=f"mx{j}")
                nc.vector.reduce_max(mx[:], seg, axis=mybir.AxisListType.X)
                sub = work.tile([P, NBLK * H2], F32, tag=f"sub{j}")
                sub3 = sub[:].rearrange("p (b h) -> p b h", h=H2)
                nc.vector.tensor_tensor(out=sub3, in0=seg,
                                        in1=mx[:].to_broadcast([P, NBLK, H2]),
                                        op=mybir.AluOpType.subtract)
                e = work.tile([P, NBLK * H2], F32, tag=f"e{j}")
                nc.scalar.activation(e[:], sub[:], AF.Exp)
                sm = small.tile([P, NBLK], F32, tag=f"sm{j}")
                nc.vector.reduce_sum(sm[:], e[:].rearrange("p (b h) -> p b h", h=H2),
                                     axis=mybir.AxisListType.X)
                rec = small.tile([P, NBLK], F32, tag=f"rec{j}")
                nc.vector.reciprocal(rec[:], sm[:])
                soft = work.tile([P, NBLK * H2], F32, tag=f"soft{j}")
                nc.vector.tensor_tensor(
                    out=soft[:].rearrange("p (b h) -> p b h", h=H2),
                    in0=e[:].rearrange("p (b h) -> p b h", h=H2),
                    in1=rec[:].to_broadcast([P, NBLK, H2]),
                    op=mybir.AluOpType.mult)
                x = work.tile([P, NBLK * H2], F32, tag=f"ez{j}")
                nc.scalar.activation(x[:], soft[:], AF.Exp)
                ez.append(x)

            # z = z_ex + s2 * (exp(sm1) + 0.1 * s1 * exp(sm2)), all blocks at once
            t1 = work.tile([P, NBLK * H2], F32, tag="t1")
            nc.vector.tensor_mul(t1[:], s1_sb[:], ez[1][:])
            t2 = work.tile([P, NBLK * H2], F32, tag="t2")
            nc.vector.tensor_scalar_mul(t2[:], t1[:], 0.1)
            zenn = work.tile([P, NBLK * H2], F32, tag="zenn")
            nc.vector.tensor_add(zenn[:], ez[0][:], t2[:])
            t3 = work.tile([P, NBLK * H2], F32, tag="t3")
            nc.vector.tensor_mul(t3[:], s2_sb[:], zenn[:])
            nc.vector.tensor_tensor(
                out=zall_f32[:].rearrange("p (b h) -> p b h", h=H2),
                in0=zc3[:, :, 0:H2],
                in1=t3[:].rearrange("p (b h) -> p b h", h=H2),
                op=mybir.AluOpType.add)
            nc.vector.tensor_copy(zall_bf[:], zall_f32[:])

            nc.sync.dma_start(z_bf.rearrange("(b p) h -> p b h", p=P), zall_bf[:])
            nc.sync.dma_start(z_f32.rearrange("(b p) h -> p b h", p=P), zall_f32[:])
    nc.compile()
    return nc


def _build_l4():
    nc = bacc.Bacc("TRN2", target_bir_lowering=False, debug=False,
                   num_devices=NCORES)
    zt = nc.dram_tensor("zt", [H2, N], BF16, kind="ExternalInput").ap()
    zt_my = nc.dram_tensor("zt_my", [H2, RS], BF16, kind="ExternalInput").ap()
    out = nc.dram_tensor("out", [RS, N], F32, kind="ExternalOutput").ap()

    with tile.TileContext(nc) as tc, ExitStack() as ctx:
        sb = ctx.enter_context(tc.tile_pool(name="sb", bufs=1))
        stg = ctx.enter_context(tc.tile_pool(name="stg", bufs=2))
        ps = ctx.enter_context(tc.tile_pool(name="ps", bufs=8, space="PSUM"))

        zt_sb = sb.tile([H2, N], BF16)
        nc.sync.dma_start(zt_sb[:], zt[:])
        ztm_sb = sb.tile([H2, RS], BF16)
        nc.sync.dma_start(ztm_sb[:], zt_my[:])

        for m in range(NBLK):
            stage = stg.tile([P, N], F32)
            for n in range(NCOL):
                acc = ps.tile([P, 512], F32)
                nc.tensor.matmul(acc[:], lhsT=ztm_sb[:, bass.ts(m, P)],
                                 rhs=zt_sb[:, bass.ts(n, 512)],
                                 start=True, stop=True)
                if n % 3 == 2:
                    nc.scalar.copy(stage[:, bass.ts(n, 512)], acc[:])
                else:
                    nc.vector.tensor_copy(stage[:, bass.ts(n, 512)], acc[:])
            nc.sync.dma_start(out[m * P:(m + 1) * P, :], stage[:])
    nc.compile()
    return nc


# --------------------------------------------------------------------------
# host-side sharding prep
# --------------------------------------------------------------------------

def _densify_at(adj_rows, adj_cols, adj_val):
    """Materialize per-core AT_shard = A_shard.T as [NCH, 128, RS] bf16
    (chunk n holds source rows n*128..n*128+127, columns = local dest rows)."""
    r = np.asarray(adj_rows)
    c = np.asarray(adj_cols)
    v = np.asarray(adj_val).astype(np.float32)
    out = []
    for core in CORE_IDS:
        sel = (r // RS) == core
        a = np.zeros((N, RS), np.float32)
        np.add.at(a, (c[sel], r[sel] - core * RS), v[sel])
        out.append(np.ascontiguousarray(a.reshape(NCH, P, RS).astype(NPBF16)))
    return out


# --------------------------------------------------------------------------
# entry point
# --------------------------------------------------------------------------

def kernel(features, adj_rows, adj_cols, adj_val, W0, W1, W2, W3,
           sample_1, sample_2, _debug=None):
    features = np.asarray(features, np.float32)
    W0 = np.asarray(W0, np.float32)
    wcat = np.ascontiguousarray(
        np.concatenate([np.asarray(W1), np.asarray(W2), np.asarray(W3)],
                       axis=1).astype(np.float32))
    s1 = np.asarray(sample_1, np.float32)
    s2 = np.asarray(sample_2, np.float32)

    at_shards = _densify_at(adj_rows, adj_cols, adj_val)

    if "l1" not in _CACHE:
        _CACHE["l1"] = _build_l1()
    if "l2" not in _CACHE:
        _CACHE["l2"] = _build_spmm(is_l3=False)
    if "l3" not in _CACHE:
        _CACHE["l3"] = _build_spmm(is_l3=True)
    if "l4" not in _CACHE:
        _CACHE["l4"] = _build_l4()

    featT = np.ascontiguousarray(features.T)           # [512, 8192]

    # ---- L1: XW0 shards ----
    in_maps = [{"featT": np.ascontiguousarray(featT[:, c * RS:(c + 1) * RS]),
                "w0": W0} for c in CORE_IDS]
    r1 = run_bass_kernel_spmd(_CACHE["l1"], in_maps, CORE_IDS)
    xw0_full = np.ascontiguousarray(
        np.concatenate([r1.results[c]["xw0"] for c in CORE_IDS], axis=0))

    # ---- L2: h1 shards ----
    in_maps = [{"src": xw0_full, "at": at_shards[c]} for c in CORE_IDS]
    r2 = run_bass_kernel_spmd(_CACHE["l2"], in_maps, CORE_IDS)
    h1_full = np.ascontiguousarray(
        np.concatenate([r2.results[c]["h1"] for c in CORE_IDS], axis=0))

    # ---- L3: z shards ----
    in_maps = [{"src": h1_full, "at": at_shards[c], "wcat": wcat,
                "s1": np.ascontiguousarray(s1[c * RS:(c + 1) * RS]),
                "s2": np.ascontiguousarray(s2[c * RS:(c + 1) * RS])}
               for c in CORE_IDS]
    r3 = run_bass_kernel_spmd(_CACHE["l3"], in_maps, CORE_IDS)
    z_bf = np.concatenate([r3.results[c]["z_bf"] for c in CORE_IDS], axis=0)
    zt_bf = np.ascontiguousarray(z_bf.T)               # [16, 8192] bf16

    # ---- L4: decode ----
    in_maps = [{"zt": zt_bf,
                "zt_my": np.ascontiguousarray(zt_bf[:, c * RS:(c + 1) * RS])}
               for c in CORE_IDS]
    r4 = run_bass_kernel_spmd(_CACHE["l4"], in_maps, CORE_IDS)
    out = np.concatenate([r4.results[c]["out"] for c in CORE_IDS], axis=0)

    if _debug is not None:
        _debug["xw0"] = xw0_full
        _debug["h1"] = h1_full
        _debug["z_bf"] = z_bf
        _debug["z_f32"] = np.concatenate(
            [r3.results[c]["z_f32"] for c in CORE_IDS], axis=0)
        _debug["t_b"] = 0
    return out.reshape(-1)


# revision 8
# speedup vs baseline: 1.0572x; 1.0572x over previous
"""Trainium2 Bass kernel for nn_GCNModelCMVAE (GCN encoder + inner-product decoder).

Self-contained: hardcodes shapes/sharding. Strategy (8 NeuronCores, row-sharded):

  L1: per-core  XW0_shard = featT_shard.T @ W0              [1024, 32]
      (host passes features pre-transposed; pure layout prep)
  -- host gathers XW0 shards -> XW0_full [8192, 32]
  L2: per-core  h1_shard = relu(A_shard @ XW0) via dense bf16 matmul against
      the host-materialized adjacency slice AT_shard [8192, 1024] (A_shard.T).
      Zeros are exact in bf16 so this equals an edge-wise scatter numerically;
      lhsT = AT chunk [128src, 128dst], rhs = XW0 chunk [128src, 32].
  -- host gathers h1 shards -> h1_full [8192, 32]
  L3: per-core  sT [32, 1024] = h1.T @ AT_shard  (lhsT = h1 chunk [128,32],
      rhs = AT chunk [128, 512]), then per 128-row block:
      zcat[128,48] = matmul(lhsT=sT[:,block], rhs=[W1|W2|W3]), softmax on
      cols 16:32 / 32:48 (free-dim reductions), reparam -> z_shard [1024, 16]
      (uses spmm(A, h1@Wi) == spmm(A, h1) @ Wi)
  -- host transposes z -> zT [16, 8192] (bf16)
  L4: per-core  decode rows: out[128,512] = matmul(lhsT=zT_my[16,128],
      rhs=zT[16,512chunk]) for 8x16 tiles; DMA 32 MiB/core to HBM
      (memory roofline: ~256 MiB total output write)
"""

import numpy as np
import ml_dtypes
from contextlib import ExitStack

import concourse.bass as bass
import concourse.tile as tile
from concourse import bacc, mybir
from concourse.bass_utils import run_bass_kernel_spmd

F32 = mybir.dt.float32
BF16 = mybir.dt.bfloat16
NPBF16 = ml_dtypes.bfloat16

N = 8192
F = 512
H1 = 32
H2 = 16
NCORES = 8
RS = N // NCORES          # 1024 rows per core
P = 128
NBLK = RS // P            # 8 row-blocks per core
KCH = F // P              # 4 contraction chunks for XW0
NCH = N // P              # 64 source chunks for the spmm
NGRP = 8                  # AT DMA groups (8 chunks = 2 MiB each)
NCOL = N // 512           # 16 column chunks in decode
CORE_IDS = list(range(NCORES))

_CACHE = {}


# --------------------------------------------------------------------------
# kernel builders
# --------------------------------------------------------------------------

def _build_l1():
    nc = bacc.Bacc("TRN2", target_bir_lowering=False, debug=False,
                   num_devices=NCORES)
    featT = nc.dram_tensor("featT", [F, RS], F32, kind="ExternalInput").ap()
    w0 = nc.dram_tensor("w0", [F, H1], F32, kind="ExternalInput").ap()
    xw0 = nc.dram_tensor("xw0", [P, NBLK * H1], F32, kind="ExternalOutput").ap()

    with tile.TileContext(nc) as tc, ExitStack() as ctx:
        sb = ctx.enter_context(tc.tile_pool(name="sb", bufs=1))
        ps = ctx.enter_context(tc.tile_pool(name="ps", bufs=4, space="PSUM"))

        w0_sb = sb.tile([P, KCH * H1], F32)
        for k in range(KCH):
            nc.sync.dma_start(w0_sb[:, bass.ts(k, H1)],
                              w0[k * P:(k + 1) * P, :])
        ft = []
        for k in range(KCH):
            t = sb.tile([P, RS], F32, tag=f"ft{k}")
            nc.sync.dma_start(t[:], featT[k * P:(k + 1) * P, :])
            ft.append(t)

        out_sb = sb.tile([P, NBLK * H1], F32)
        for m in range(NBLK):
            acc = ps.tile([P, H1], F32)
            for k in range(KCH):
                nc.tensor.matmul(acc[:], lhsT=ft[k][:, bass.ts(m, P)],
                                 rhs=w0_sb[:, bass.ts(k, H1)],
                                 start=(k == 0), stop=(k == KCH - 1))
            nc.vector.tensor_copy(out_sb[:, bass.ts(m, H1)], acc[:])
        nc.sync.dma_start(xw0[:], out_sb[:])
    nc.compile()
    return nc


def _build_spmm(is_l3):
    """L2: h1_shard = relu(A_shard @ x) row-major.
    L3: sT = (A_shard @ h1).T via transposed orientation, then the z tail.
    Both stream the dense AT_shard [8192, 1024] bf16 in NGRP groups."""
    nc = bacc.Bacc("TRN2", target_bir_lowering=False, debug=False,
                   num_devices=NCORES)
    src = nc.dram_tensor("src", [P, NCH * H1], F32, kind="ExternalInput").ap()
    at = nc.dram_tensor("at", [NCH, P, RS], BF16, kind="ExternalInput").ap()
    if is_l3:
        wcat = nc.dram_tensor("wcat", [H1, 3 * H2], F32, kind="ExternalInput").ap()
        s1 = nc.dram_tensor("s1", [P, NBLK * H2], F32, kind="ExternalInput").ap()
        s2 = nc.dram_tensor("s2", [P, NBLK * H2], F32, kind="ExternalInput").ap()
        z_bf = nc.dram_tensor("z_bf", [P, NBLK * H2], BF16, kind="ExternalOutput").ap()
        z_f32 = nc.dram_tensor("z_f32", [P, NBLK * H2], F32, kind="ExternalOutput").ap()
    else:
        h1 = nc.dram_tensor("h1", [P, NBLK * H1], F32, kind="ExternalOutput").ap()

    AF = mybir.ActivationFunctionType
    ch_per_grp = NCH // NGRP
    with tile.TileContext(nc) as tc, ExitStack() as ctx:
        sb = ctx.enter_context(tc.tile_pool(name="sb", bufs=1))
        atp = ctx.enter_context(tc.tile_pool(name="atp", bufs=2))
        work = ctx.enter_context(tc.tile_pool(name="work", bufs=3))
        small = ctx.enter_context(tc.tile_pool(name="small", bufs=4))
        ps = ctx.enter_context(tc.tile_pool(name="ps", bufs=1, space="PSUM"))
        ps2 = ctx.enter_context(tc.tile_pool(name="ps2", bufs=2, space="PSUM"))

        # x [8192, 32] f32 -> SBUF [128, 64, 32] (chunk n on cols ts(n, 32)), cast bf16
        x_f = sb.tile([P, NCH * H1], F32)
        nc.sync.dma_start(x_f[:], src[:])
        x_bf = sb.tile([P, NCH * H1], BF16)
        nc.vector.tensor_copy(x_bf[:], x_f[:])

        if is_l3:
            # accumulate sT [32, 1024] across all 64 chunks (2 psum halves)
            acc = [ps.tile([H1, 512], F32, tag=f"sT{h}", name=f"sT{h}") for h in range(2)]
        else:
            acc = [ps.tile([P, H1], F32, tag=f"h1_{m}", name=f"h1acc{m}") for m in range(NBLK)]

        for g in range(NGRP):
            at_g = atp.tile([P, ch_per_grp * RS], BF16)
            nc.sync.dma_start(at_g[:], at[g * ch_per_grp:(g + 1) * ch_per_grp]
                              .rearrange("c p r -> p c r"))
            for ci in range(ch_per_grp):
                n = g * ch_per_grp + ci
                first, last = (n == 0), (n == NCH - 1)
                if is_l3:
                    for h in range(2):
                        nc.tensor.matmul(
                            acc[h][:],
                            lhsT=x_bf[:, bass.ts(n, H1)],
                            rhs=at_g[:, ci * RS + h * 512: ci * RS + (h + 1) * 512],
                            start=first, stop=last)
                else:
                    for m in range(NBLK):
                        nc.tensor.matmul(
                            acc[m][:],
                            lhsT=at_g[:, ci * RS + m * P: ci * RS + (m + 1) * P],
                            rhs=x_bf[:, bass.ts(n, H1)],
                            start=first, stop=last)

        if not is_l3:
            out_sb = sb.tile([P, NBLK * H1], F32)
            for m in range(NBLK):
                nc.scalar.activation(out_sb[:, bass.ts(m, H1)], acc[m][:], AF.Relu)
            nc.sync.dma_start(h1[:], out_sb[:])
        else:
            st_sb = sb.tile([H1, RS], F32)
            for h in range(2):
                nc.vector.tensor_copy(st_sb[:, h * 512:(h + 1) * 512], acc[h][:])

            wcat_sb = sb.tile([H1, 3 * H2], F32)
            nc.sync.dma_start(wcat_sb[:], wcat[:])
            s1_sb = sb.tile([P, NBLK * H2], F32)
            nc.sync.dma_start(s1_sb[:], s1[:])
            s2_sb = sb.tile([P, NBLK * H2], F32)
            nc.sync.dma_start(s2_sb[:], s2[:])
            zall_bf = sb.tile([P, NBLK * H2], BF16)
            zall_f32 = sb.tile([P, NBLK * H2], F32)

            # zcat for all 8 blocks into one psum tile [128, 8*48] (1.5 KB/bank)
            zps = ps2.tile([P, NBLK * 3 * H2], F32)
            for b in range(NBLK):
                nc.tensor.matmul(zps[:, bass.ts(b, 3 * H2)],
                                 lhsT=st_sb[:, bass.ts(b, P)],
                                 rhs=wcat_sb[:], start=True, stop=True)
            zc = sb.tile([P, NBLK * 3 * H2], F32)
            nc.vector.tensor_copy(zc[:], zps[:])
            zc3 = zc[:].rearrange("p (b j) -> p b j", j=3 * H2)

            # all-blocks-fused softmax -> exp for segs 1 (z_en) and 2 (z_he)
            ez = []
            for j in (1, 2):
                seg = zc3[:, :, j * H2:(j + 1) * H2]        # [P, 8, 16]
                mx = small.tile([P, NBLK], F32, tag=f"mx{j}")
                nc.vector.reduce_max(mx[:], seg, axis=mybir.AxisListType.X)
                sub = work.tile([P, NBLK * H2], F32, tag=f"sub{j}")
                sub3 = sub[:].rearrange("p (b h) -> p b h", h=H2)
                nc.vector.tensor_tensor(out=sub3, in0=seg,
                                        in1=mx[:].to_broadcast([P, NBLK, H2]),
                                        op=mybir.AluOpType.subtract)
                e = work.tile([P, NBLK * H2], F32, tag=f"e{j}")
                nc.scalar.activation(e[:], sub[:], AF.Exp)
                sm = small.tile([P, NBLK], F32, tag=f"sm{j}")
                nc.vector.reduce_sum(sm[:], e[:].rearrange("p (b h) -> p b h", h=H2),
                                     axis=mybir.AxisListType.X)
                rec = small.tile([P, NBLK], F32, tag=f"rec{j}")
                nc.vector.reciprocal(rec[:], sm[:])
                soft = work.tile([P, NBLK * H2], F32, tag=f"soft{j}")
                nc.vector.tensor_tensor(
                    out=soft[:].rearrange("p (b h) -> p b h", h=H2),
                    in0=e[:].rearrange("p (b h) -> p b h", h=H2),
                    in1=rec[:].to_broadcast([P, NBLK, H2]),
                    op=mybir.AluOpType.mult)
                x = work.tile([P, NBLK * H2], F32, tag=f"ez{j}")
                nc.scalar.activation(x[:], soft[:], AF.Exp)
                ez.append(x)

            # z = z_ex + s2 * (exp(sm1) + 0.1 * s1 * exp(sm2)), all blocks at once
            t1 = work.tile([P, NBLK * H2], F32, tag="t1")
            nc.vector.tensor_mul(t1[:], s1_sb[:], ez[1][:])
            t2 = work.tile([P, NBLK * H2], F32, tag="t2")
            nc.vector.tensor_scalar_mul(t2[:], t1[:], 0.1)
            zenn = work.tile([P, NBLK * H2], F32, tag="zenn")
            nc.vector.tensor_add(zenn[:], ez[0][:], t2[:])
            t3 = work.tile([P, NBLK * H2], F32, tag="t3")
            nc.vector.tensor_mul(t3[:], s2_sb[:], zenn[:])
            nc.vector.tensor_tensor(
                out=zall_f32[:].rearrange("p (b h) -> p b h", h=H2),
                in0=zc3[:, :, 0:H2],
                in1=t3[:].rearrange("p (b h) -> p b h", h=H2),
                op=mybir.AluOpType.add)
            nc.vector.tensor_copy(zall_bf[:], zall_f32[:])

            nc.sync.dma_start(z_bf[:], zall_bf[:])
            nc.sync.dma_start(z_f32[:], zall_f32[:])
    nc.compile()
    return nc


def _build_l4():
    nc = bacc.Bacc("TRN2", target_bir_lowering=False, debug=False,
                   num_devices=NCORES)
    zt = nc.dram_tensor("zt", [H2, N], BF16, kind="ExternalInput").ap()
    zt_my = nc.dram_tensor("zt_my", [H2, RS], BF16, kind="ExternalInput").ap()
    out = nc.dram_tensor("out", [RS, N], F32, kind="ExternalOutput").ap()

    with tile.TileContext(nc) as tc, ExitStack() as ctx:
        sb = ctx.enter_context(tc.tile_pool(name="sb", bufs=1))
        stg = ctx.enter_context(tc.tile_pool(name="stg", bufs=2))
        ps = ctx.enter_context(tc.tile_pool(name="ps", bufs=8, space="PSUM"))

        zt_sb = sb.tile([H2, N], BF16)
        nc.sync.dma_start(zt_sb[:], zt[:])
        ztm_sb = sb.tile([H2, RS], BF16)
        nc.sync.dma_start(ztm_sb[:], zt_my[:])

        for m in range(NBLK):
            stage = stg.tile([P, N], F32)
            for n in range(NCOL):
                acc = ps.tile([P, 512], F32)
                nc.tensor.matmul(acc[:], lhsT=ztm_sb[:, bass.ts(m, P)],
                                 rhs=zt_sb[:, bass.ts(n, 512)],
                                 start=True, stop=True)
                if n % 3 == 2:
                    nc.scalar.copy(stage[:, bass.ts(n, 512)], acc[:])
                else:
                    nc.vector.tensor_copy(stage[:, bass.ts(n, 512)], acc[:])
            for q in range(4):
                nc.sync.dma_start(out[m * P:(m + 1) * P, q * 2048:(q + 1) * 2048],
                                  stage[:, bass.ts(q, 2048)])
    nc.compile()
    return nc


# --------------------------------------------------------------------------
# host-side sharding prep
# --------------------------------------------------------------------------

def _densify_at(adj_rows, adj_cols, adj_val):
    """Materialize per-core AT_shard = A_shard.T as [NCH, 128, RS] bf16
    (chunk n holds source rows n*128..n*128+127, columns = local dest rows)."""
    r = np.asarray(adj_rows)
    c = np.asarray(adj_cols)
    v = np.asarray(adj_val).astype(np.float32)
    out = []
    for core in CORE_IDS:
        sel = (r // RS) == core
        a = np.zeros((N, RS), np.float32)
        np.add.at(a, (c[sel], r[sel] - core * RS), v[sel])
        out.append(np.ascontiguousarray(a.reshape(NCH, P, RS).astype(NPBF16)))
    return out


# --------------------------------------------------------------------------
# entry point
# --------------------------------------------------------------------------

def kernel(features, adj_rows, adj_cols, adj_val, W0, W1, W2, W3,
           sample_1, sample_2, _debug=None):
    features = np.asarray(features, np.float32)
    W0 = np.asarray(W0, np.float32)
    wcat = np.ascontiguousarray(
        np.concatenate([np.asarray(W1), np.asarray(W2), np.asarray(W3)],
                       axis=1).astype(np.float32))
    s1 = np.asarray(sample_1, np.float32)
    s2 = np.asarray(sample_2, np.float32)

    at_shards = _densify_at(adj_rows, adj_cols, adj_val)

    if "l1" not in _CACHE:
        _CACHE["l1"] = _build_l1()
    if "l2" not in _CACHE:
        _CACHE["l2"] = _build_spmm(is_l3=False)
    if "l3" not in _CACHE:
        _CACHE["l3"] = _build_spmm(is_l3=True)
    if "l4" not in _CACHE:
        _CACHE["l4"] = _build_l4()

    featT = np.ascontiguousarray(features.T)           # [512, 8192]

    # ---- L1: XW0 shards (out: [128, NBLK, H1] = (p, m, f) per core) ----
    in_maps = [{"featT": np.ascontiguousarray(featT[:, c * RS:(c + 1) * RS]),
                "w0": W0} for c in CORE_IDS]
    r1 = run_bass_kernel_spmd(_CACHE["l1"], in_maps, CORE_IDS)
    # global chunk n = c*NBLK + m, so [p, n, f] layout = concat along axis 1
    xw0_pnf = np.ascontiguousarray(np.concatenate(
        [r1.results[c]["xw0"].reshape(P, NBLK, H1) for c in CORE_IDS], axis=1)
        .reshape(P, NCH * H1))

    # ---- L2: h1 shards ----
    in_maps = [{"src": xw0_pnf, "at": at_shards[c]} for c in CORE_IDS]
    r2 = run_bass_kernel_spmd(_CACHE["l2"], in_maps, CORE_IDS)
    h1_pnf = np.ascontiguousarray(np.concatenate(
        [r2.results[c]["h1"].reshape(P, NBLK, H1) for c in CORE_IDS], axis=1)
        .reshape(P, NCH * H1))

    # ---- L3: z shards ----
    def _pbh(a):  # [RS, H2] row-major -> [P, NBLK*H2] (p, b, h)
        return np.ascontiguousarray(
            a.reshape(NBLK, P, H2).transpose(1, 0, 2).reshape(P, NBLK * H2))

    in_maps = [{"src": h1_pnf, "at": at_shards[c], "wcat": wcat,
                "s1": _pbh(s1[c * RS:(c + 1) * RS]),
                "s2": _pbh(s2[c * RS:(c + 1) * RS])}
               for c in CORE_IDS]
    r3 = run_bass_kernel_spmd(_CACHE["l3"], in_maps, CORE_IDS)

    def _un_pbh(a):  # [P, NBLK*H2] (p, b, h) -> [RS, H2] row-major
        return a.reshape(P, NBLK, H2).transpose(1, 0, 2).reshape(RS, H2)

    z_bf = np.concatenate(
        [_un_pbh(r3.results[c]["z_bf"]) for c in CORE_IDS], axis=0)
    zt_bf = np.ascontiguousarray(z_bf.T)               # [16, 8192] bf16

    # ---- L4: decode ----
    in_maps = [{"zt": zt_bf,
                "zt_my": np.ascontiguousarray(zt_bf[:, c * RS:(c + 1) * RS])}
               for c in CORE_IDS]
    r4 = run_bass_kernel_spmd(_CACHE["l4"], in_maps, CORE_IDS)
    out = np.concatenate([r4.results[c]["out"] for c in CORE_IDS], axis=0)

    if _debug is not None:
        _debug["xw0"] = xw0_pnf.reshape(P, NCH, H1).transpose(1, 0, 2).reshape(N, H1)
        _debug["h1"] = h1_pnf.reshape(P, NCH, H1).transpose(1, 0, 2).reshape(N, H1)
        _debug["z_bf"] = z_bf
        _debug["z_f32"] = np.concatenate(
            [_un_pbh(r3.results[c]["z_f32"]) for c in CORE_IDS], axis=0)
        _debug["t_b"] = 0
    return out.reshape(-1)
